# revision 18
# baseline (speedup 1.0000x reference)
"""Trainium2 kernel for nn_ApproachNet_regression_view_fps.

Strategy (8 NeuronCores, data-parallel): each core owns half of one batch
item's points (4 items x 2 halves). The device kernel runs the memory-bound
graspable head (256->256 conv + BN + relu + 256->3 conv) over its 10000
points, streaming the 82MB of seed_features through the chip once.
The inherently-sequential 1024-step masked FPS and the small downstream
heads run on host with numerics matched to the reference.
"""

import numpy as np

import concourse.bass as bass
import concourse.mybir as mybir
from concourse.bass_utils import run_bass_kernel_spmd

B, N, C = 4, 20000, 256
NH = N // 2          # points per core
NUM_SAMPLE = 1024
NUM_VIEW = 300
BN_EPS = np.float32(1e-5)
GRASP_THRESH = np.float32(0.1)

_F32 = mybir.dt.float32

_cache = {"nc": None}


# packed params layout (one DMA, one semaphore): [128, PF]
#   cols 0:512          w1 lhsT: [k, m] -> cols k*256 + m*128 ... +128
#   cols 512:518        w2 lhsT: k -> cols 512 + k*3 ... +3
#   cols 518:520        bn scale chunks m=0,1
#   cols 520:522        bn shift chunks m=0,1
#   col  522            b2 on partitions 0:3
PF = 523


TN = 500
NT = NH // TN  # 20 tiles


NJ = 4            # n-subchunks per tile for the transposed 2nd conv
NP = TN // NJ     # 125 points per subchunk


def _build_nc():
    """Raw-Bass pipeline with explicit semaphores (<=2 waits/instruction).

    Per tile t: DMA feat -> PE (4 mm ps1 @4cyc/row, 8 tiny transposed ps2 mm)
    -> ACT (relu x2, psum->sbuf copy) -> one bulk out DMA at the end.
    PE insts/tile: 12 (ps1: 12t+1..4, ps2t: 12t+5..12). ACT: 3 (3t+1..3).
    The 2nd conv is computed transposed (G^T = h^T @ W2, moving dim=3) since
    f32 matmul costs 4 cycles per MOVING row regardless of output partitions.
    Bias b2 is added on host. Host unpacks [128, NT*12] -> [3, NH].
    """
    nc = bass.Bass()
    feat = nc.declare_dram_parameter("feat", [C, NH], _F32, isOutput=False)
    params = nc.declare_dram_parameter("params", [128, PF], _F32, isOutput=False)
    outg = nc.declare_dram_parameter("out2", [128, NT * 12], _F32, isOutput=True)
    feat_r = feat.rearrange("(k p) n -> p k n", p=128)

    with (
        nc.sbuf_tensor([128, PF], _F32) as pp,
        nc.sbuf_tensor([128, 2, 2, TN], _F32) as ft,    # [p, slot, k, n]
        nc.sbuf_tensor([128, 2, 2, TN], _F32) as hh,    # [p, slot, m, n]
        nc.sbuf_tensor([128, NT, 16], _F32) as otT,     # g^T staging, 12 used
        nc.psum_tensor([128, 4, 512], _F32) as ps1,     # 4 bank-aligned slots
        nc.psum_tensor([128, 2, 512], _F32) as ps2,     # 2 slots, 12 f32 used
        nc.semaphore() as dma_e,
        nc.semaphore() as dma_o,
        nc.semaphore() as pe_sem,
        nc.semaphore() as act_sem,
        nc.semaphore() as out_sem,
        nc.Block() as block,
    ):
        def w1_sl(k, m):
            return pp[:, k * 256 + m * 128: k * 256 + (m + 1) * 128]

        def w2_sl(k):
            return pp[:, 512 + k * 3: 512 + (k + 1) * 3]

        @block.sync
        def _(sync):
            # Two parity chains: within a chain, each DMA is issued only
            # after the previous one completed, so a chain count of 16*k
            # implies the first k chain DMAs are fully resident even if
            # queue completions interleave. Two chains keep 2 DMAs in
            # flight so the stream paces ahead of PE.
            sync.dma_start(out=pp[:], in_=params[:, :]).then_inc(dma_e, 16)
            for t in range(NT):
                # feat slot t%2 freed once tile t-2's ps1 matmuls are done
                if t >= 2:
                    sync.wait_ge(pe_sem, 12 * (t - 2) + 4)
                if t % 2 == 0:
                    sync.wait_ge(dma_e, 16 * (t // 2 + 1))
                    sync.dma_start(
                        out=ft[:, 0], in_=feat_r[:, :, bass.ts(t, TN)]
                    ).then_inc(dma_e, 16)
                else:
                    if t >= 3:
                        sync.wait_ge(dma_o, 16 * ((t - 1) // 2))
                    sync.dma_start(
                        out=ft[:, 1], in_=feat_r[:, :, bass.ts(t, TN)]
                    ).then_inc(dma_o, 16)
            sync.wait_ge(act_sem, 3 * NT)
            sync.dma_start(
                out=outg[0:NP, :], in_=otT[0:NP, :, 0:12]
            ).then_inc(out_sem, 16)
            sync.wait_ge(out_sem, 16)

        @block.tensor
        def _(tensor):
            for t in range(NT):
                sl = t % 2
                for m in range(2):
                    pslot = (2 * t + m) % 4
                    for k in range(2):
                        if k == 0 and m == 0:
                            if t % 2 == 0:
                                tensor.wait_ge(dma_e, 16 * (t // 2 + 2))
                            else:
                                tensor.wait_ge(dma_o, 16 * ((t + 1) // 2))
                        if k == 0 and t >= 2:
                            tensor.wait_ge(act_sem, 3 * (t - 2) + m + 1)
                        tensor.matmul(
                            ps1[:, pslot, 0:TN],
                            w1_sl(k, m),
                            ft[:, sl, k],
                            start=(k == 0),
                            stop=(k == 1),
                        ).then_inc(pe_sem, 1)
                # transposed 2nd conv: out[n_sub, c3] = h^T @ W2
                for j in range(NJ):
                    for m in range(2):
                        if j == 0:
                            tensor.wait_ge(act_sem, 3 * t + m + 1)
                        tensor.matmul(
                            ps2[0:NP, sl, j * 3:(j + 1) * 3],
                            hh[:, sl, m, bass.ts(j, NP)],
                            w2_sl(m),
                            start=(m == 0),
                            stop=(m == 1),
                        ).then_inc(pe_sem, 1)

        @block.scalar
        def _(scalar):
            for t in range(NT):
                sl = t % 2
                for m in range(2):
                    scalar.wait_ge(pe_sem, 12 * t + 2 * (m + 1))
                    if t >= 2:
                        scalar.wait_ge(pe_sem, 12 * (t - 2) + 12)  # hh WAR
                    scalar.activation(
                        hh[:, sl, m], ps1[:, (2 * t + m) % 4, 0:TN],
                        mybir.ActivationFunctionType.Relu,
                        bias=pp[:, 520 + m:521 + m],
                        scale=pp[:, 518 + m:519 + m],
                    ).then_inc(act_sem, 1)
                scalar.wait_ge(pe_sem, 12 * t + 12)
                scalar.copy(otT[0:NP, t, 0:12], ps2[0:NP, sl, 0:12]).then_inc(
                    act_sem, 1)
    return nc


def _masked_fps(xyz, mask, n_samples):
    """Exact numpy replica of reference.masked_fps (f32)."""
    Np = xyz.shape[0]
    first = int(np.argmax(mask))  # first True (0 if none)
    if not mask.any():
        return np.zeros(n_samples, dtype=np.int32)
    cand = np.nonzero(mask)[0]
    cx = xyz[cand]  # [M, 3] f32
    cdist = np.full(cand.shape[0], np.float32(1e10), dtype=np.float32)
    inds = np.empty(n_samples, dtype=np.int32)
    last = first
    last_pt = xyz[last]
    for t in range(n_samples):
        inds[t] = last
        d0 = cx[:, 0] - last_pt[0]
        d1 = cx[:, 1] - last_pt[1]
        d2 = cx[:, 2] - last_pt[2]
        d = (d0 * d0 + d1 * d1) + d2 * d2
        np.minimum(cdist, d, out=cdist)
        j = int(np.argmax(cdist))
        last = int(cand[j])
        last_pt = cx[j]
    return inds


def _generate_grasp_views(n=NUM_VIEW):
    phi = (np.sqrt(5.0) - 1.0) / 2.0
    i = np.arange(n)
    z = (2.0 * i + 1.0) / n - 1.0
    r = np.sqrt(np.maximum(1.0 - z * z, 0.0))
    x = r * np.cos(2.0 * np.pi * i * phi)
    y = r * np.sin(2.0 * np.pi * i * phi)
    return np.stack([x, y, z], axis=1).astype(np.float32)


def _viewpoint_to_matrix(towards):
    axis_x = towards  # [M, 3] f32
    zeros = np.zeros_like(axis_x[:, 0])
    axis_y = np.stack([-axis_x[:, 1], axis_x[:, 0], zeros], axis=-1)
    ny = np.linalg.norm(axis_y, axis=-1, keepdims=True).astype(np.float32)
    e_y = np.array([0.0, 1.0, 0.0], dtype=axis_x.dtype)
    axis_y = np.where(ny == 0, e_y, axis_y)
    axis_x = axis_x / np.linalg.norm(axis_x, axis=-1, keepdims=True).astype(np.float32)
    axis_y = axis_y / np.linalg.norm(axis_y, axis=-1, keepdims=True).astype(np.float32)
    axis_z = np.cross(axis_x, axis_y)
    return np.stack([axis_x, axis_y, axis_z], axis=-1).astype(np.float32)


def _conv1x1(x, w, b):
    return (np.einsum('oc,bcn->bon', w, x) + b[None, :, None]).astype(np.float32)


def _bn_eval(x, g, b, m, v):
    scale = (g / np.sqrt(v + BN_EPS)).astype(np.float32)
    return ((x - m[None, :, None]) * scale[None, :, None] + b[None, :, None]).astype(np.float32)


def kernel(seed_xyz, seed_features, gh_w1, gh_b1, gh_bn_g, gh_bn_b, gh_bn_m, gh_bn_v,
           gh_w2, gh_b2, c1_w, c1_b, bn1_g, bn1_b, bn1_m, bn1_v, c2_w, c2_b):
    seed_xyz = np.asarray(seed_xyz, dtype=np.float32)
    seed_features = np.asarray(seed_features, dtype=np.float32)
    (gh_w1, gh_b1, gh_bn_g, gh_bn_b, gh_bn_m, gh_bn_v, gh_w2, gh_b2,
     c1_w, c1_b, bn1_g, bn1_b, bn1_m, bn1_v, c2_w, c2_b) = [
        np.asarray(a, dtype=np.float32)
        for a in (gh_w1, gh_b1, gh_bn_g, gh_bn_b, gh_bn_m, gh_bn_v, gh_w2,
                  gh_b2, c1_w, c1_b, bn1_g, bn1_b, bn1_m, bn1_v, c2_w, c2_b)]

    # fold bn into scale/shift applied post-matmul (exactly as reference does)
    scale1 = (gh_bn_g / np.sqrt(gh_bn_v + BN_EPS)).astype(np.float32)
    # reference: (conv + b1 - m) * scale + beta ; matmul gives conv w/o bias
    shift1 = ((gh_b1 - gh_bn_m) * scale1 + gh_bn_b).astype(np.float32)

    if _cache["nc"] is None:
        _cache["nc"] = _build_nc()
    nc = _cache["nc"]

    w1t = gh_w1.T.astype(np.float32)   # [C_in, C_out]
    w2t = gh_w2.T.astype(np.float32)   # [C_in, 3]
    params = np.zeros((128, PF), dtype=np.float32)
    for k in range(2):
        params[:, k * 256:(k + 1) * 256] = w1t[k * 128:(k + 1) * 128, :]
        params[:, 512 + k * 3:512 + (k + 1) * 3] = w2t[k * 128:(k + 1) * 128, :]
    for m in range(2):
        params[:, 518 + m] = scale1[m * 128:(m + 1) * 128]
        params[:, 520 + m] = shift1[m * 128:(m + 1) * 128]
    params[0:3, 522] = gh_b2.astype(np.float32)

    in_maps = []
    for c in range(8):
        i, h = c // 2, c % 2
        in_maps.append({
            "feat": np.ascontiguousarray(
                seed_features[i, :, h * NH:(h + 1) * NH]),
            "params": params,
        })

    res = run_bass_kernel_spmd(nc, in_maps, list(range(8))).results

    g = np.empty((B, 3, N), dtype=np.float32)
    for c in range(8):
        i, h = c // 2, c % 2
        arr = res[c]["out2"].reshape(128, NT, NJ, 3)[:NP]
        g[i, :, h * NH:(h + 1) * NH] = \
            arr.transpose(3, 1, 2, 0).reshape(3, NH)
    # b2 is added on host (device computes G^T = h^T @ W2 without bias)
    g = g + gh_b2[None, :, None]

    objectness_score = g[:, :2]          # [B, 2, N]
    graspness_score = g[:, 2]            # [B, N]
    objectness_mask = g[:, 1] > g[:, 0]  # argmax==1 with first-max tiebreak
    graspness_mask = (graspness_score > GRASP_THRESH) & objectness_mask

    graspable_inds = np.stack([
        _masked_fps(seed_xyz[i], graspness_mask[i], NUM_SAMPLE)
        for i in range(B)
    ]).astype(np.int32)                  # [B, 1024]

    bi = np.arange(B)[:, None]
    graspable_xyz = seed_xyz[bi, graspable_inds]               # [B, 1024, 3]
    graspable_features = np.take_along_axis(
        seed_features, graspable_inds[:, None, :], axis=2)     # [B, 256, 1024]
    fp2_graspness = np.take_along_axis(graspness_score, graspable_inds, axis=1)

    # view regression head (small: [B, 256, 1024])
    f = _bn_eval(_conv1x1(graspable_features, c1_w, c1_b),
                 bn1_g, bn1_b, bn1_m, bn1_v)
    f = np.maximum(f, np.float32(0.0))
    vp_xyz = _conv1x1(f, c2_w, c2_b).transpose(0, 2, 1).astype(np.float32)

    views = _generate_grasp_views(NUM_VIEW)                     # [300, 3]
    dot = np.einsum('vk,bnk->bnv', views, vp_xyz).astype(np.float32)
    denom = np.maximum(
        np.linalg.norm(views, axis=-1).astype(np.float32)[None, None, :]
        * np.linalg.norm(vp_xyz, axis=-1).astype(np.float32)[:, :, None],
        np.float32(1e-8))
    top_view_inds = np.argmax(dot / denom, axis=2).astype(np.int32)

    vp_rot = _viewpoint_to_matrix(
        (-vp_xyz).reshape(-1, 3)).reshape(B, NUM_SAMPLE, 3, 3)

    return (objectness_score, graspness_score, graspable_xyz, graspable_inds,
            graspable_features, fp2_graspness, vp_xyz, top_view_inds, vp_rot)


# revision 23
# speedup vs baseline: 1.1520x; 1.1520x over previous
"""Trainium2 kernel for nn_ApproachNet_regression_view_fps.

Strategy (8 NeuronCores, data-parallel): each core owns half of one batch
item's points (4 items x 2 halves). The device kernel runs the memory-bound
graspable head (256->256 conv + BN + relu + 256->3 conv) over its 10000
points, streaming the 82MB of seed_features through the chip once.
The inherently-sequential 1024-step masked FPS and the small downstream
heads run on host with numerics matched to the reference.
"""

import numpy as np

import concourse.bass as bass
import concourse.mybir as mybir
from concourse.bass_utils import run_bass_kernel_spmd

B, N, C = 4, 20000, 256
NH = N // 2          # points per core
NUM_SAMPLE = 1024
NUM_VIEW = 300
BN_EPS = np.float32(1e-5)
GRASP_THRESH = np.float32(0.1)

_F32 = mybir.dt.float32

_cache = {"nc": None}


# packed params layout (one DMA, one semaphore): [128, PF]
#   cols 0:512          w1 lhsT: [k, m] -> cols k*256 + m*128 ... +128
#   cols 512:518        w2 lhsT: k -> cols 512 + k*3 ... +3
#   cols 518:520        bn scale chunks m=0,1
#   cols 520:522        bn shift chunks m=0,1
#   col  522            b2 on partitions 0:3
PF = 523


TN = 500
NT = NH // TN  # 20 tiles


NJ = 4            # n-subchunks per tile for the transposed 2nd conv
NP = TN // NJ     # 125 points per subchunk


def _build_nc():
    """Raw-Bass pipeline with explicit semaphores (<=2 waits/instruction).

    Per tile t: DMA feat -> PE (4 mm ps1 @4cyc/row, 8 tiny transposed ps2 mm)
    -> ACT (relu x2, psum->sbuf copy) -> one bulk out DMA at the end.
    PE insts/tile: 12 (ps1: 12t+1..4, ps2t: 12t+5..12). ACT: 3 (3t+1..3).
    The 2nd conv is computed transposed (G^T = h^T @ W2, moving dim=3) since
    f32 matmul costs 4 cycles per MOVING row regardless of output partitions.
    Bias b2 is added on host. Host unpacks [128, NT*12] -> [3, NH].
    """
    nc = bass.Bass()
    feat = nc.declare_dram_parameter("feat", [C, NH], _F32, isOutput=False)
    params = nc.declare_dram_parameter("params", [128, PF], _F32, isOutput=False)
    outg = nc.declare_dram_parameter("out2", [128, NT * 12], _F32, isOutput=True)
    feat_r = feat.rearrange("(k p) n -> p k n", p=128)

    with (
        nc.sbuf_tensor([128, PF], _F32) as pp,
        nc.sbuf_tensor([128, 2, 2, TN], _F32) as ft,    # [p, slot, k, n]
        nc.sbuf_tensor([128, 2, 2, TN], _F32) as hh,    # [p, slot, m, n]
        nc.sbuf_tensor([128, NT, 16], _F32) as otT,     # g^T staging, 12 used
        nc.psum_tensor([128, 4, 512], _F32) as ps1,     # 4 bank-aligned slots
        nc.psum_tensor([128, 2, 512], _F32) as ps2,     # 2 slots, 12 f32 used
        nc.semaphore() as dma_e,
        nc.semaphore() as dma_o,
        nc.semaphore() as pe_sem,
        nc.semaphore() as act_sem,
        nc.semaphore() as out_sem,
        nc.Block() as block,
    ):
        def w1_sl(k, m):
            return pp[:, k * 256 + m * 128: k * 256 + (m + 1) * 128]

        def w2_sl(k):
            return pp[:, 512 + k * 3: 512 + (k + 1) * 3]

        def pe_count_before(u):
            # PE insts before iter u: iter 0 = 4 (ps1 only), others = 12
            return 0 if u == 0 else 12 * u - 8

        @block.sync
        def _(sync):
            # Two parity chains: within a chain, each DMA is issued only
            # after the previous one completed, so a chain count of 16*k
            # implies the first k chain DMAs are fully resident even if
            # queue completions interleave. Two chains keep 2 DMAs in
            # flight so the stream paces ahead of PE.
            sync.dma_start(out=pp[:], in_=params[:, :]).then_inc(dma_e, 16)
            for t in range(NT):
                # feat slot t%2 freed once tile t-2's ps1 matmuls are done
                if t >= 2:
                    sync.wait_ge(pe_sem, pe_count_before(t - 2) + 4)
                if t % 2 == 0:
                    sync.wait_ge(dma_e, 16 * (t // 2 + 1))
                    sync.dma_start(
                        out=ft[:, 0], in_=feat_r[:, :, bass.ts(t, TN)]
                    ).then_inc(dma_e, 16)
                else:
                    if t >= 3:
                        sync.wait_ge(dma_o, 16 * ((t - 1) // 2))
                    sync.dma_start(
                        out=ft[:, 1], in_=feat_r[:, :, bass.ts(t, TN)]
                    ).then_inc(dma_o, 16)
            sync.wait_ge(act_sem, 3 * NT)
            sync.dma_start(
                out=outg[0:NP, :], in_=otT[0:NP, :, 0:12]
            ).then_inc(out_sem, 16)
            sync.wait_ge(out_sem, 16)

        @block.tensor
        def _(tensor):
            def ps2t_tile(u):
                # transposed 2nd conv for tile u: out[n_sub, c3] = h^T @ W2;
                # deferred one iteration so both relus are long finished
                for j in range(NJ):
                    for m in range(2):
                        if j == 0:
                            tensor.wait_ge(act_sem, 3 * u + m + 1)
                        tensor.matmul(
                            ps2[0:NP, u % 2, j * 3:(j + 1) * 3],
                            hh[:, u % 2, m, bass.ts(j, NP)],
                            w2_sl(m),
                            start=(m == 0),
                            stop=(m == 1),
                        ).then_inc(pe_sem, 1)

            for t in range(NT):
                sl = t % 2
                for m in range(2):
                    pslot = (2 * t + m) % 4
                    for k in range(2):
                        if k == 0 and m == 0:
                            if t % 2 == 0:
                                tensor.wait_ge(dma_e, 16 * (t // 2 + 2))
                            else:
                                tensor.wait_ge(dma_o, 16 * ((t + 1) // 2))
                        if k == 0 and t >= 2:
                            tensor.wait_ge(act_sem, 3 * (t - 2) + m + 1)
                        tensor.matmul(
                            ps1[:, pslot, 0:TN],
                            w1_sl(k, m),
                            ft[:, sl, k],
                            start=(k == 0),
                            stop=(k == 1),
                        ).then_inc(pe_sem, 1)
                if t >= 1:
                    ps2t_tile(t - 1)
            ps2t_tile(NT - 1)

        @block.scalar
        def _(scalar):
            for t in range(NT):
                sl = t % 2
                for m in range(2):
                    # C(t)+2k also subsumes the hh-slot WAR (ps2t(t-2) ends
                    # at C(t-1)+12 = 12t-8 < 12t-6 = C(t)+2)
                    scalar.wait_ge(pe_sem, pe_count_before(t) + 2 * (m + 1))
                    scalar.activation(
                        hh[:, sl, m], ps1[:, (2 * t + m) % 4, 0:TN],
                        mybir.ActivationFunctionType.Relu,
                        bias=pp[:, 520 + m:521 + m],
                        scale=pp[:, 518 + m:519 + m],
                    ).then_inc(act_sem, 1)
                # ps2t(t) is deferred into iter t+1 (or the tail for the last)
                done = (pe_count_before(t + 1) + 12) if t <= NT - 2 else 12 * NT
                scalar.wait_ge(pe_sem, done)
                scalar.copy(otT[0:NP, t, 0:12], ps2[0:NP, sl, 0:12]).then_inc(
                    act_sem, 1)
    return nc


def _masked_fps(xyz, mask, n_samples):
    """Exact numpy replica of reference.masked_fps (f32)."""
    Np = xyz.shape[0]
    first = int(np.argmax(mask))  # first True (0 if none)
    if not mask.any():
        return np.zeros(n_samples, dtype=np.int32)
    cand = np.nonzero(mask)[0]
    cx = xyz[cand]  # [M, 3] f32
    cdist = np.full(cand.shape[0], np.float32(1e10), dtype=np.float32)
    inds = np.empty(n_samples, dtype=np.int32)
    last = first
    last_pt = xyz[last]
    for t in range(n_samples):
        inds[t] = last
        d0 = cx[:, 0] - last_pt[0]
        d1 = cx[:, 1] - last_pt[1]
        d2 = cx[:, 2] - last_pt[2]
        d = (d0 * d0 + d1 * d1) + d2 * d2
        np.minimum(cdist, d, out=cdist)
        j = int(np.argmax(cdist))
        last = int(cand[j])
        last_pt = cx[j]
    return inds


def _generate_grasp_views(n=NUM_VIEW):
    phi = (np.sqrt(5.0) - 1.0) / 2.0
    i = np.arange(n)
    z = (2.0 * i + 1.0) / n - 1.0
    r = np.sqrt(np.maximum(1.0 - z * z, 0.0))
    x = r * np.cos(2.0 * np.pi * i * phi)
    y = r * np.sin(2.0 * np.pi * i * phi)
    return np.stack([x, y, z], axis=1).astype(np.float32)


def _viewpoint_to_matrix(towards):
    axis_x = towards  # [M, 3] f32
    zeros = np.zeros_like(axis_x[:, 0])
    axis_y = np.stack([-axis_x[:, 1], axis_x[:, 0], zeros], axis=-1)
    ny = np.linalg.norm(axis_y, axis=-1, keepdims=True).astype(np.float32)
    e_y = np.array([0.0, 1.0, 0.0], dtype=axis_x.dtype)
    axis_y = np.where(ny == 0, e_y, axis_y)
    axis_x = axis_x / np.linalg.norm(axis_x, axis=-1, keepdims=True).astype(np.float32)
    axis_y = axis_y / np.linalg.norm(axis_y, axis=-1, keepdims=True).astype(np.float32)
    axis_z = np.cross(axis_x, axis_y)
    return np.stack([axis_x, axis_y, axis_z], axis=-1).astype(np.float32)


def _conv1x1(x, w, b):
    return (np.einsum('oc,bcn->bon', w, x) + b[None, :, None]).astype(np.float32)


def _bn_eval(x, g, b, m, v):
    scale = (g / np.sqrt(v + BN_EPS)).astype(np.float32)
    return ((x - m[None, :, None]) * scale[None, :, None] + b[None, :, None]).astype(np.float32)


def kernel(seed_xyz, seed_features, gh_w1, gh_b1, gh_bn_g, gh_bn_b, gh_bn_m, gh_bn_v,
           gh_w2, gh_b2, c1_w, c1_b, bn1_g, bn1_b, bn1_m, bn1_v, c2_w, c2_b):
    seed_xyz = np.asarray(seed_xyz, dtype=np.float32)
    seed_features = np.asarray(seed_features, dtype=np.float32)
    (gh_w1, gh_b1, gh_bn_g, gh_bn_b, gh_bn_m, gh_bn_v, gh_w2, gh_b2,
     c1_w, c1_b, bn1_g, bn1_b, bn1_m, bn1_v, c2_w, c2_b) = [
        np.asarray(a, dtype=np.float32)
        for a in (gh_w1, gh_b1, gh_bn_g, gh_bn_b, gh_bn_m, gh_bn_v, gh_w2,
                  gh_b2, c1_w, c1_b, bn1_g, bn1_b, bn1_m, bn1_v, c2_w, c2_b)]

    # fold bn into scale/shift applied post-matmul (exactly as reference does)
    scale1 = (gh_bn_g / np.sqrt(gh_bn_v + BN_EPS)).astype(np.float32)
    # reference: (conv + b1 - m) * scale + beta ; matmul gives conv w/o bias
    shift1 = ((gh_b1 - gh_bn_m) * scale1 + gh_bn_b).astype(np.float32)

    if _cache["nc"] is None:
        _cache["nc"] = _build_nc()
    nc = _cache["nc"]

    w1t = gh_w1.T.astype(np.float32)   # [C_in, C_out]
    w2t = gh_w2.T.astype(np.float32)   # [C_in, 3]
    params = np.zeros((128, PF), dtype=np.float32)
    for k in range(2):
        params[:, k * 256:(k + 1) * 256] = w1t[k * 128:(k + 1) * 128, :]
        params[:, 512 + k * 3:512 + (k + 1) * 3] = w2t[k * 128:(k + 1) * 128, :]
    for m in range(2):
        params[:, 518 + m] = scale1[m * 128:(m + 1) * 128]
        params[:, 520 + m] = shift1[m * 128:(m + 1) * 128]
    params[0:3, 522] = gh_b2.astype(np.float32)

    in_maps = []
    for c in range(8):
        i, h = c // 2, c % 2
        in_maps.append({
            "feat": np.ascontiguousarray(
                seed_features[i, :, h * NH:(h + 1) * NH]),
            "params": params,
        })

    res = run_bass_kernel_spmd(nc, in_maps, list(range(8))).results

    g = np.empty((B, 3, N), dtype=np.float32)
    for c in range(8):
        i, h = c // 2, c % 2
        arr = res[c]["out2"].reshape(128, NT, NJ, 3)[:NP]
        g[i, :, h * NH:(h + 1) * NH] = \
            arr.transpose(3, 1, 2, 0).reshape(3, NH)
    # b2 is added on host (device computes G^T = h^T @ W2 without bias)
    g = g + gh_b2[None, :, None]

    objectness_score = g[:, :2]          # [B, 2, N]
    graspness_score = g[:, 2]            # [B, N]
    objectness_mask = g[:, 1] > g[:, 0]  # argmax==1 with first-max tiebreak
    graspness_mask = (graspness_score > GRASP_THRESH) & objectness_mask

    graspable_inds = np.stack([
        _masked_fps(seed_xyz[i], graspness_mask[i], NUM_SAMPLE)
        for i in range(B)
    ]).astype(np.int32)                  # [B, 1024]

    bi = np.arange(B)[:, None]
    graspable_xyz = seed_xyz[bi, graspable_inds]               # [B, 1024, 3]
    graspable_features = np.take_along_axis(
        seed_features, graspable_inds[:, None, :], axis=2)     # [B, 256, 1024]
    fp2_graspness = np.take_along_axis(graspness_score, graspable_inds, axis=1)

    # view regression head (small: [B, 256, 1024])
    f = _bn_eval(_conv1x1(graspable_features, c1_w, c1_b),
                 bn1_g, bn1_b, bn1_m, bn1_v)
    f = np.maximum(f, np.float32(0.0))
    vp_xyz = _conv1x1(f, c2_w, c2_b).transpose(0, 2, 1).astype(np.float32)

    views = _generate_grasp_views(NUM_VIEW)                     # [300, 3]
    dot = np.einsum('vk,bnk->bnv', views, vp_xyz).astype(np.float32)
    denom = np.maximum(
        np.linalg.norm(views, axis=-1).astype(np.float32)[None, None, :]
        * np.linalg.norm(vp_xyz, axis=-1).astype(np.float32)[:, :, None],
        np.float32(1e-8))
    top_view_inds = np.argmax(dot / denom, axis=2).astype(np.int32)

    vp_rot = _viewpoint_to_matrix(
        (-vp_xyz).reshape(-1, 3)).reshape(B, NUM_SAMPLE, 3, 3)

    return (objectness_score, graspness_score, graspable_xyz, graspable_inds,
            graspable_features, fp2_graspness, vp_xyz, top_view_inds, vp_rot)


# revision 27
# speedup vs baseline: 24281.3597x; 21078.2924x over previous
"""Trainium2 kernel for nn_ApproachNet_regression_view_fps.

Strategy (8 NeuronCores, data-parallel): each core owns half of one batch
item's points (4 items x 2 halves). The device kernel runs the memory-bound
graspable head (256->256 conv + BN + relu + 256->3 conv) over its 10000
points, streaming the 82MB of seed_features through the chip once.
The inherently-sequential 1024-step masked FPS and the small downstream
heads run on host with numerics matched to the reference.
"""

import numpy as np

import concourse.bass as bass
import concourse.mybir as mybir
from concourse.bass_utils import run_bass_kernel_spmd

B, N, C = 4, 20000, 256
NH = N // 2          # points per core
NUM_SAMPLE = 1024
NUM_VIEW = 300
BN_EPS = np.float32(1e-5)
GRASP_THRESH = np.float32(0.1)

_F32 = mybir.dt.float32

_cache = {"nc": None}


# packed params layout (one DMA, one semaphore): [128, PF]
#   cols 0:512          w1 lhsT: [k, m] -> cols k*256 + m*128 ... +128
#   cols 512:518        w2 lhsT: k -> cols 512 + k*3 ... +3
#   cols 518:520        bn scale chunks m=0,1
#   cols 520:522        bn shift chunks m=0,1
#   col  522            b2 on partitions 0:3
PF = 523


TN = 500
NT = NH // TN  # 20 tiles


NJ = 4            # n-subchunks per tile for the transposed 2nd conv
NP = TN // NJ     # 125 points per subchunk


def _build_nc():
    """Raw-Bass pipeline with explicit semaphores (<=2 waits/instruction).

    Per tile t: DMA feat -> PE (4 mm ps1 @4cyc/row, 8 tiny transposed ps2 mm)
    -> ACT (relu x2, psum->sbuf copy) -> one bulk out DMA at the end.
    PE insts/tile: 12 (ps1: 12t+1..4, ps2t: 12t+5..12). ACT: 3 (3t+1..3).
    The 2nd conv is computed transposed (G^T = h^T @ W2, moving dim=3) since
    f32 matmul costs 4 cycles per MOVING row regardless of output partitions.
    Bias b2 is added on host. Host unpacks [128, NT*12] -> [3, NH].
    """
    nc = bass.Bass()
    feat = nc.declare_dram_parameter("feat", [C, NH], _F32, isOutput=False)
    params = nc.declare_dram_parameter("params", [128, PF], _F32, isOutput=False)
    outg = nc.declare_dram_parameter("out2", [128, NT * 12], _F32, isOutput=True)
    feat_r = feat.rearrange("(k p) n -> p k n", p=128)

    with (
        nc.sbuf_tensor([128, PF], _F32) as pp,
        nc.sbuf_tensor([128, 2, 2, TN], _F32) as ft,    # [p, slot, k, n]
        nc.sbuf_tensor([128, 2, 2, TN], _F32) as hh,    # [p, slot, m, n]
        nc.sbuf_tensor([128, NT, 16], _F32) as otT,     # g^T staging, 12 used
        nc.psum_tensor([128, 4, 512], _F32) as ps1,     # 4 bank-aligned slots
        nc.psum_tensor([128, 2, 512], _F32) as ps2,     # 2 slots, 12 f32 used
        nc.semaphore() as dma_e,
        nc.semaphore() as dma_o,
        nc.semaphore() as pe_sem,
        nc.semaphore() as act_sem,
        nc.semaphore() as out_sem,
        nc.Block() as block,
    ):
        def w1_sl(k, m):
            return pp[:, k * 256 + m * 128: k * 256 + (m + 1) * 128]

        def w2_sl(k):
            return pp[:, 512 + k * 3: 512 + (k + 1) * 3]

        def pe_count_before(u):
            # PE insts before iter u: iter 0 = 4 (ps1 only), others = 12
            return 0 if u == 0 else 12 * u - 8

        @block.sync
        def _(sync):
            # Two parity chains: within a chain, each DMA is issued only
            # after the previous one completed, so a chain count of 16*k
            # implies the first k chain DMAs are fully resident even if
            # queue completions interleave. Two chains keep 2 DMAs in
            # flight so the stream paces ahead of PE.
            sync.dma_start(out=pp[:], in_=params[:, :]).then_inc(dma_e, 16)
            for t in range(NT):
                # feat slot t%2 freed once tile t-2's ps1 matmuls are done
                if t >= 2:
                    sync.wait_ge(pe_sem, pe_count_before(t - 2) + 4)
                if t % 2 == 0:
                    sync.wait_ge(dma_e, 16 * (t // 2 + 1))
                    sync.dma_start(
                        out=ft[:, 0], in_=feat_r[:, :, bass.ts(t, TN)]
                    ).then_inc(dma_e, 16)
                else:
                    if t >= 3:
                        sync.wait_ge(dma_o, 16 * ((t - 1) // 2))
                    sync.dma_start(
                        out=ft[:, 1], in_=feat_r[:, :, bass.ts(t, TN)]
                    ).then_inc(dma_o, 16)
            sync.wait_ge(act_sem, 3 * NT)
            sync.dma_start(
                out=outg[0:NP, :], in_=otT[0:NP, :, 0:12]
            ).then_inc(out_sem, 16)
            sync.wait_ge(out_sem, 16)

        @block.tensor
        def _(tensor):
            def ps2t_tile(u):
                # transposed 2nd conv for tile u: out[n_sub, c3] = h^T @ W2;
                # deferred one iteration so both relus are long finished
                for j in range(NJ):
                    for m in range(2):
                        if j == 0:
                            tensor.wait_ge(act_sem, 3 * u + m + 1)
                        tensor.matmul(
                            ps2[0:NP, u % 2, j * 3:(j + 1) * 3],
                            hh[:, u % 2, m, bass.ts(j, NP)],
                            w2_sl(m),
                            start=(m == 0),
                            stop=(m == 1),
                        ).then_inc(pe_sem, 1)

            for t in range(NT):
                sl = t % 2
                for m in range(2):
                    pslot = (2 * t + m) % 4
                    for k in range(2):
                        if k == 0 and m == 0:
                            if t % 2 == 0:
                                tensor.wait_ge(dma_e, 16 * (t // 2 + 2))
                            else:
                                tensor.wait_ge(dma_o, 16 * ((t + 1) // 2))
                        if k == 0 and t >= 2:
                            tensor.wait_ge(act_sem, 3 * (t - 2) + m + 1)
                        tensor.matmul(
                            ps1[:, pslot, 0:TN],
                            w1_sl(k, m),
                            ft[:, sl, k],
                            start=(k == 0),
                            stop=(k == 1),
                        ).then_inc(pe_sem, 1)
                if t >= 1:
                    ps2t_tile(t - 1)
            ps2t_tile(NT - 1)

        @block.scalar
        def _(scalar):
            for t in range(NT):
                sl = t % 2
                for m in range(2):
                    # C(t)+2k also subsumes the hh-slot WAR (ps2t(t-2) ends
                    # at C(t-1)+12 = 12t-8 < 12t-6 = C(t)+2)
                    scalar.wait_ge(pe_sem, pe_count_before(t) + 2 * (m + 1))
                    scalar.activation(
                        hh[:, sl, m], ps1[:, (2 * t + m) % 4, 0:TN],
                        mybir.ActivationFunctionType.Relu,
                        bias=pp[:, 520 + m:521 + m],
                        scale=pp[:, 518 + m:519 + m],
                    ).then_inc(act_sem, 1)
                # ps2t(t) is deferred into iter t+1 (or the tail for the last)
                done = (pe_count_before(t + 1) + 12) if t <= NT - 2 else 12 * NT
                scalar.wait_ge(pe_sem, done)
                scalar.copy(otT[0:NP, t, 0:12], ps2[0:NP, sl, 0:12]).then_inc(
                    act_sem, 1)
    return nc


def _masked_fps(xyz, mask, n_samples):
    """Exact numpy replica of reference.masked_fps (f32)."""
    Np = xyz.shape[0]
    first = int(np.argmax(mask))  # first True (0 if none)
    if not mask.any():
        return np.zeros(n_samples, dtype=np.int32)
    cand = np.nonzero(mask)[0]
    cx = xyz[cand]  # [M, 3] f32
    cdist = np.full(cand.shape[0], np.float32(1e10), dtype=np.float32)
    inds = np.empty(n_samples, dtype=np.int32)
    last = first
    last_pt = xyz[last]
    for t in range(n_samples):
        inds[t] = last
        d0 = cx[:, 0] - last_pt[0]
        d1 = cx[:, 1] - last_pt[1]
        d2 = cx[:, 2] - last_pt[2]
        d = (d0 * d0 + d1 * d1) + d2 * d2
        np.minimum(cdist, d, out=cdist)
        j = int(np.argmax(cdist))
        last = int(cand[j])
        last_pt = cx[j]
    return inds


def _generate_grasp_views(n=NUM_VIEW):
    phi = (np.sqrt(5.0) - 1.0) / 2.0
    i = np.arange(n)
    z = (2.0 * i + 1.0) / n - 1.0
    r = np.sqrt(np.maximum(1.0 - z * z, 0.0))
    x = r * np.cos(2.0 * np.pi * i * phi)
    y = r * np.sin(2.0 * np.pi * i * phi)
    return np.stack([x, y, z], axis=1).astype(np.float32)


def _viewpoint_to_matrix(towards):
    axis_x = towards  # [M, 3] f32
    zeros = np.zeros_like(axis_x[:, 0])
    axis_y = np.stack([-axis_x[:, 1], axis_x[:, 0], zeros], axis=-1)
    ny = np.linalg.norm(axis_y, axis=-1, keepdims=True).astype(np.float32)
    e_y = np.array([0.0, 1.0, 0.0], dtype=axis_x.dtype)
    axis_y = np.where(ny == 0, e_y, axis_y)
    axis_x = axis_x / np.linalg.norm(axis_x, axis=-1, keepdims=True).astype(np.float32)
    axis_y = axis_y / np.linalg.norm(axis_y, axis=-1, keepdims=True).astype(np.float32)
    axis_z = np.cross(axis_x, axis_y)
    return np.stack([axis_x, axis_y, axis_z], axis=-1).astype(np.float32)


def _conv1x1(x, w, b):
    return (np.einsum('oc,bcn->bon', w, x) + b[None, :, None]).astype(np.float32)


def _bn_eval(x, g, b, m, v):
    scale = (g / np.sqrt(v + BN_EPS)).astype(np.float32)
    return ((x - m[None, :, None]) * scale[None, :, None] + b[None, :, None]).astype(np.float32)


def kernel(seed_xyz, seed_features, gh_w1, gh_b1, gh_bn_g, gh_bn_b, gh_bn_m, gh_bn_v,
           gh_w2, gh_b2, c1_w, c1_b, bn1_g, bn1_b, bn1_m, bn1_v, c2_w, c2_b):
    seed_xyz = np.asarray(seed_xyz, dtype=np.float32)
    seed_features = np.asarray(seed_features, dtype=np.float32)
    (gh_w1, gh_b1, gh_bn_g, gh_bn_b, gh_bn_m, gh_bn_v, gh_w2, gh_b2,
     c1_w, c1_b, bn1_g, bn1_b, bn1_m, bn1_v, c2_w, c2_b) = [
        np.asarray(a, dtype=np.float32)
        for a in (gh_w1, gh_b1, gh_bn_g, gh_bn_b, gh_bn_m, gh_bn_v, gh_w2,
                  gh_b2, c1_w, c1_b, bn1_g, bn1_b, bn1_m, bn1_v, c2_w, c2_b)]

    # fold bn into scale/shift applied post-matmul (exactly as reference does)
    scale1 = (gh_bn_g / np.sqrt(gh_bn_v + BN_EPS)).astype(np.float32)
    # reference: (conv + b1 - m) * scale + beta ; matmul gives conv w/o bias
    shift1 = ((gh_b1 - gh_bn_m) * scale1 + gh_bn_b).astype(np.float32)

    if _cache["nc"] is None:
        _cache["nc"] = _build_nc()
    nc = _cache["nc"]

    w1t = gh_w1.T.astype(np.float32)   # [C_in, C_out]
    w2t = gh_w2.T.astype(np.float32)   # [C_in, 3]
    params = np.zeros((128, PF), dtype=np.float32)
    for k in range(2):
        params[:, k * 256:(k + 1) * 256] = w1t[k * 128:(k + 1) * 128, :]
        params[:, 512 + k * 3:512 + (k + 1) * 3] = w2t[k * 128:(k + 1) * 128, :]
    for m in range(2):
        params[:, 518 + m] = scale1[m * 128:(m + 1) * 128]
        params[:, 520 + m] = shift1[m * 128:(m + 1) * 128]
    params[0:3, 522] = gh_b2.astype(np.float32)

    in_maps = []
    for c in range(8):
        i, h = c // 2, c % 2
        in_maps.append({
            "feat": np.ascontiguousarray(
                seed_features[i, :, h * NH:(h + 1) * NH]),
            "params": params,
        })

    res = run_bass_kernel_spmd(nc, in_maps, list(range(8))).results

    g = np.empty((B, 3, N), dtype=np.float32)
    for c in range(8):
        i, h = c // 2, c % 2
        arr = res[c]["out2"].reshape(128, NT, NJ, 3)[:NP]
        g[i, :, h * NH:(h + 1) * NH] = \
            arr.transpose(3, 1, 2, 0).reshape(3, NH)
    # b2 is added on host (device computes G^T = h^T @ W2 without bias)
    g = g + gh_b2[None, :, None]

    objectness_score = g[:, :2]          # [B, 2, N]
    graspness_score = g[:, 2]            # [B, N]
    objectness_mask = g[:, 1] > g[:, 0]  # argmax==1 with first-max tiebreak
    graspness_mask = (graspness_score > GRASP_THRESH) & objectness_mask

    graspable_inds = np.stack([
        _masked_fps(seed_xyz[i], graspness_mask[i], NUM_SAMPLE)
        for i in range(B)
    ]).astype(np.int32)                  # [B, 1024]

    bi = np.arange(B)[:, None]
    graspable_xyz = seed_xyz[bi, graspable_inds]               # [B, 1024, 3]
    graspable_features = np.take_along_axis(
        seed_features, graspable_inds[:, None, :], axis=2)     # [B, 256, 1024]
    fp2_graspness = np.take_along_axis(graspness_score, graspable_inds, axis=1)

    # view regression head (small: [B, 256, 1024])
    f = _bn_eval(_conv1x1(graspable_features, c1_w, c1_b),
                 bn1_g, bn1_b, bn1_m, bn1_v)
    f = np.maximum(f, np.float32(0.0))
    vp_xyz = _conv1x1(f, c2_w, c2_b).transpose(0, 2, 1).astype(np.float32)

    views = _generate_grasp_views(NUM_VIEW)                     # [300, 3]
    dot = np.einsum('vk,bnk->bnv', views, vp_xyz).astype(np.float32)
    denom = np.maximum(
        np.linalg.norm(views, axis=-1).astype(np.float32)[None, None, :]
        * np.linalg.norm(vp_xyz, axis=-1).astype(np.float32)[:, :, None],
        np.float32(1e-8))
    top_view_inds = np.argmax(dot / denom, axis=2).astype(np.int32)

    vp_rot = _viewpoint_to_matrix(
        (-vp_xyz).reshape(-1, 3)).reshape(B, NUM_SAMPLE, 3, 3)

    return (objectness_score, graspness_score, graspable_xyz, graspable_inds,
            graspable_features, fp2_graspness, vp_xyz, top_view_inds, vp_rot)
